# revision 1
# baseline (speedup 1.0000x reference)
"""Distributed flash-style InfoNCE loss kernel for Trainium2 (8 NeuronCores).

Problem: two 3-layer MLP encoders (X and Y) -> [B,B] critic scores ->
InfoNCE MI lower bound:  loss = -(log(B) + mean_i(scores[i,i] - logsumexp_j scores[i,j]))

Sharding: data-parallel over batch rows (1024 rows/core) for both encoders;
zY^T is AllGathered (embed dim is only 128) and each core computes its own
1024 rows of the critic + per-row logsumexp, never materializing the [B,B]
matrix (exp+row-sum happens straight out of PSUM with a fused accumulator).

All activations live in transposed layout ([features on partitions, rows on
free dim]) so every weight matrix loads from DRAM in its natural [K, M]
layout as the stationary matmul operand, and the only transpose needed is
the input (done on the PE with an identity matmul). Matmuls run as
float32r (full PE rate, ~fp32 accuracy).

Per-core output: [128, 8] tile of (pos - lse) per row; the host sums and
applies log(B)/mean. The kernel is rank-oblivious: the positive-pair
diagonal is computed from the core's LOCAL zX/zY shards (same global rows),
so all 8 cores run an identical program on different data.
"""

import numpy as np

import concourse.bacc as bacc
import concourse.bass as bass
import concourse.mybir as mybir
import concourse.tile as tile
from concourse.bass_utils import run_bass_kernel_spmd
from concourse.masks import make_identity

# Problem shapes (hardcoded; kernel.py must be self-contained).
B, NX, NY, HID, EMB = 8192, 512, 512, 1024, 128
NCORES = 8
BS = B // NCORES          # 1024 rows per core
P = 128                   # SBUF partitions
MB = BS // P              # 8 row-blocks per core
NCH = BS // 512           # 2 psum chunks of 512 per 1024 rows
F32 = mybir.dt.float32
F32R = mybir.dt.float32r
AX = mybir.AxisListType
ALU = mybir.AluOpType
ACT = mybir.ActivationFunctionType


def _fr(ap):
    return ap if ap.dtype == F32R else ap.bitcast(F32R)


def _load_bias(nc, pool, name, handle, nblk):
    """[nblk*128] DRAM bias -> [128, nblk] SBUF tile (per-partition layout)."""
    t = pool.tile([P, nblk], F32, name=name, tag=name)
    nc.sync.dma_start(t, handle.ap().rearrange("(m p) -> p m", p=P))
    return t


def _encoder(nc, tc, pools, data, W0, b0t, W1, b1t, W2, b2t, nin_k, prefix):
    """One 3-layer MLP in transposed-activation layout.

    data: [BS, nin_k*128] ExternalInput. Returns zT: [128, BS] SBUF tile.
    """
    const, wpool, xpool, big, zpool, inpool, psum_mm, psum_t = pools
    ident = const["ident"]

    # ---- transpose input: [BS, NIN] -> xt [128, nin_k*BS] (col = kb*BS + row)
    xt = xpool.tile([P, 4 * BS], F32R, name="xt", tag="xt")
    for rb in range(MB):
        xin = inpool.tile([P, nin_k * P], F32, name="xin", tag="xin")
        nc.sync.dma_start(xin, data.ap()[rb * P:(rb + 1) * P, :])
        for kb in range(nin_k):
            pt = psum_t.tile([P, P], F32, name="pt", tag="pt")
            nc.tensor.transpose(pt, xin[:, kb * P:(kb + 1) * P], ident)
            nc.vector.tensor_copy(xt[:, kb * BS + rb * P:kb * BS + (rb + 1) * P], pt)

    # ---- L0: h1T = relu(W0.T @ xT + b0)   [8 x BS]
    w0 = []
    for kb in range(nin_k):
        w = wpool.tile([P, HID], F32R, name=f"{prefix}w0_{kb}", tag="w")
        nc.sync.dma_start(w, W0.ap()[kb * P:(kb + 1) * P, :].bitcast(F32R))
        w0.append(w)
    h1 = big.tile([P, 8 * BS], F32R, name=f"{prefix}h1", tag="big")
    for m in range(8):
        for ch in range(NCH):
            ps = psum_mm.tile([P, 512], F32, name="ps", tag="ps")
            for kb in range(nin_k):
                nc.tensor.matmul(
                    ps, _fr(w0[kb][:, m * P:(m + 1) * P]),
                    _fr(xt[:, kb * BS + ch * 512:kb * BS + ch * 512 + 512]),
                    start=(kb == 0), stop=(kb == nin_k - 1))
            nc.vector.tensor_scalar(
                out=h1[:, m * BS + ch * 512:m * BS + ch * 512 + 512], in0=ps,
                scalar1=b0t[:, m:m + 1], scalar2=0.0, op0=ALU.add, op1=ALU.max)

    # ---- L1: h2T = relu(W1.T @ h1T + b1)  [8 x BS]
    w1 = []
    for kb in range(8):
        w = wpool.tile([P, HID], F32R, name=f"{prefix}w1_{kb}", tag="w")
        nc.sync.dma_start(w, W1.ap()[kb * P:(kb + 1) * P, :].bitcast(F32R))
        w1.append(w)
    h2 = big.tile([P, 8 * BS], F32R, name=f"{prefix}h2", tag="big")
    for m in range(8):
        for ch in range(NCH):
            ps = psum_mm.tile([P, 512], F32, name="ps", tag="ps")
            for kb in range(8):
                nc.tensor.matmul(
                    ps, _fr(w1[kb][:, m * P:(m + 1) * P]),
                    _fr(h1[:, kb * BS + ch * 512:kb * BS + ch * 512 + 512]),
                    start=(kb == 0), stop=(kb == 7))
            nc.vector.tensor_scalar(
                out=h2[:, m * BS + ch * 512:m * BS + ch * 512 + 512], in0=ps,
                scalar1=b1t[:, m:m + 1], scalar2=0.0, op0=ALU.add, op1=ALU.max)

    # ---- L2 (linear head): zT = W2.T @ h2T + b2  [128, BS]
    # W2 is [HID, 128]; pack its 8 k-tiles into one [128, 1024] tile.
    w2 = wpool.tile([P, HID], F32R, name=f"{prefix}w2", tag="w")
    for kb in range(8):
        nc.sync.dma_start(w2[:, kb * P:(kb + 1) * P],
                          W2.ap()[kb * P:(kb + 1) * P, :].bitcast(F32R))
    zt = zpool.tile([P, BS], F32R, name=f"{prefix}zt", tag=f"{prefix}zt")
    for ch in range(NCH):
        ps = psum_mm.tile([P, 512], F32, name="ps", tag="ps")
        for kb in range(8):
            nc.tensor.matmul(
                ps, _fr(w2[:, kb * P:(kb + 1) * P]),
                _fr(h2[:, kb * BS + ch * 512:kb * BS + ch * 512 + 512]),
                start=(kb == 0), stop=(kb == 7))
        nc.vector.tensor_scalar(
            out=zt[:, ch * 512:ch * 512 + 512], in0=ps,
            scalar1=b2t[:, 0:1], scalar2=None, op0=ALU.add)
    return zt


def build(nrep=1):
    nc = bacc.Bacc("TRN2", target_bir_lowering=False, debug=False,
                   num_devices=NCORES)

    dX = nc.dram_tensor("dataX", [BS, NX], F32, kind="ExternalInput")
    dY = nc.dram_tensor("dataY", [BS, NY], F32, kind="ExternalInput")
    Wx0 = nc.dram_tensor("Wx0", [NX, HID], F32, kind="ExternalInput")
    bx0 = nc.dram_tensor("bx0", [HID], F32, kind="ExternalInput")
    Wx1 = nc.dram_tensor("Wx1", [HID, HID], F32, kind="ExternalInput")
    bx1 = nc.dram_tensor("bx1", [HID], F32, kind="ExternalInput")
    Wx2 = nc.dram_tensor("Wx2", [HID, EMB], F32, kind="ExternalInput")
    bx2 = nc.dram_tensor("bx2", [EMB], F32, kind="ExternalInput")
    Wy0 = nc.dram_tensor("Wy0", [NY, HID], F32, kind="ExternalInput")
    by0 = nc.dram_tensor("by0", [HID], F32, kind="ExternalInput")
    Wy1 = nc.dram_tensor("Wy1", [HID, HID], F32, kind="ExternalInput")
    by1 = nc.dram_tensor("by1", [HID], F32, kind="ExternalInput")
    Wy2 = nc.dram_tensor("Wy2", [HID, EMB], F32, kind="ExternalInput")
    by2 = nc.dram_tensor("by2", [EMB], F32, kind="ExternalInput")
    out = nc.dram_tensor("out", [P, MB], F32, kind="ExternalOutput")

    with tile.TileContext(nc) as tc:
        from contextlib import ExitStack
        with ExitStack() as ctx:
            const = ctx.enter_context(tc.tile_pool(name="const", bufs=1))
            wpool = ctx.enter_context(tc.tile_pool(name="wpool", bufs=12))
            xpool = ctx.enter_context(tc.tile_pool(name="xpool", bufs=1))
            big = ctx.enter_context(tc.tile_pool(name="big", bufs=3))
            zpool = ctx.enter_context(tc.tile_pool(name="zpool", bufs=1))
            inpool = ctx.enter_context(tc.tile_pool(name="inpool", bufs=4))
            epool = ctx.enter_context(tc.tile_pool(name="epool", bufs=4))
            spool = ctx.enter_context(tc.tile_pool(name="spool", bufs=2))
            dram = ctx.enter_context(tc.tile_pool(name="dram", bufs=1, space="DRAM"))
            psum_mm = ctx.enter_context(tc.tile_pool(name="psum_mm", bufs=4, space="PSUM"))
            psum_t = ctx.enter_context(tc.tile_pool(name="psum_t", bufs=4, space="PSUM"))

            ident = const.tile([P, P], F32, name="ident", tag="ident")
            make_identity(nc, ident)
            bx0t = _load_bias(nc, const, "bx0t", bx0, 8)
            bx1t = _load_bias(nc, const, "bx1t", bx1, 8)
            bx2t = _load_bias(nc, const, "bx2t", bx2, 1)
            by0t = _load_bias(nc, const, "by0t", by0, 8)
            by1t = _load_bias(nc, const, "by1t", by1, 8)
            by2t = _load_bias(nc, const, "by2t", by2, 1)

            pools = ({"ident": ident}, wpool, xpool, big, zpool, inpool,
                     psum_mm, psum_t)

            for rep in range(nrep):
                # ---- Y encoder first, so the AllGather overlaps the X encoder.
                zyt = _encoder(nc, tc, pools, dY, Wy0, by0t, Wy1, by1t, Wy2,
                               by2t, NY // P, "y")

                zy_bounce = dram.tile([P, BS], F32, name=f"zy_bounce{rep}")
                zy_all = dram.tile([NCORES * P, BS], F32, name=f"zy_all{rep}",
                                   addr_space="Shared")
                nc.sync.dma_start(zy_bounce.bitcast(F32R), zyt)
                nc.gpsimd.collective_compute(
                    "AllGather", ALU.bypass,
                    replica_groups=[list(range(NCORES))],
                    ins=[zy_bounce.opt()], outs=[zy_all.opt()])

                # All-gathered zY^T: [128, 8192], col = rank*1024 + local row.
                zyall = big.tile([P, B], F32R, name="zyall", tag="big")
                for r in range(NCORES):
                    nc.sync.dma_start(zyall[:, r * BS:(r + 1) * BS],
                                      zy_all[r * P:(r + 1) * P, :].bitcast(F32R))

                # ---- X encoder (overlaps the collective).
                zxt = _encoder(nc, tc, pools, dX, Wx0, bx0t, Wx1, bx1t, Wx2,
                               bx2t, NX // P, "x")

                # ---- positive pairs: diag(zX_m @ zY_m^T) from LOCAL shards.
                pos_t = spool.tile([P, MB], F32, name="pos_t", tag="pos")
                for m in range(MB):
                    pd = psum_t.tile([P, P], F32, name="pd", tag="pt")
                    nc.tensor.matmul(pd, _fr(zxt[:, m * P:(m + 1) * P]),
                                     _fr(zyt[:, m * P:(m + 1) * P]),
                                     start=True, stop=True)
                    dsc = epool.tile([P, 512], F32, name="et", tag="et")
                    nc.vector.tensor_mul(dsc[:, :P], pd, ident)
                    nc.vector.reduce_sum(pos_t[:, m:m + 1], dsc[:, :P],
                                         axis=AX.X)

                # ---- critic rows + logsumexp (scores never hit SBUF).
                lse_t = spool.tile([P, MB], F32, name="lse_t", tag="lse")
                for m in range(MB):
                    sume = spool.tile([P, 2 * NCORES], F32, name="sume",
                                      tag="sume")
                    for r in range(NCORES):
                        for ch in range(NCH):
                            ps = psum_mm.tile([P, 512], F32, name="ps",
                                              tag="ps")
                            nc.tensor.matmul(
                                ps, _fr(zxt[:, m * P:(m + 1) * P]),
                                _fr(zyall[:, r * BS + ch * 512:
                                          r * BS + ch * 512 + 512]),
                                start=True, stop=True)
                            et = epool.tile([P, 512], F32, name="et", tag="et")
                            nc.scalar.activation(
                                et, ps, ACT.Exp,
                                accum_out=sume[:, r * NCH + ch:
                                               r * NCH + ch + 1])
                    tot = spool.tile([P, 1], F32, name="tot", tag="tot")
                    nc.vector.reduce_sum(tot, sume, axis=AX.X)
                    nc.scalar.activation(lse_t[:, m:m + 1], tot, ACT.Ln)

                vals = spool.tile([P, MB], F32, name="vals", tag="vals")
                nc.vector.tensor_sub(vals, pos_t, lse_t)
                nc.sync.dma_start(out.ap(), vals)

    nc.compile()
    return nc


_NC_CACHE = None


def _get_nc():
    global _NC_CACHE
    if _NC_CACHE is None:
        _NC_CACHE = build()
    return _NC_CACHE


def kernel(**inputs) -> np.ndarray:
    nc = _get_nc()
    arrs = {k: np.ascontiguousarray(np.asarray(v, dtype=np.float32))
            for k, v in inputs.items()}
    shared = {k: v for k, v in arrs.items() if k not in ("dataX", "dataY")}
    in_maps = []
    for c in range(NCORES):
        m = dict(shared)
        m["dataX"] = np.ascontiguousarray(arrs["dataX"][c * BS:(c + 1) * BS])
        m["dataY"] = np.ascontiguousarray(arrs["dataY"][c * BS:(c + 1) * BS])
        in_maps.append(m)
    res = run_bass_kernel_spmd(nc, in_maps, core_ids=list(range(NCORES)))
    vals = np.stack([res.results[c]["out"] for c in range(NCORES)])  # [8,128,8]
    total = vals.astype(np.float64).sum()
    loss = -(np.log(np.float64(B)) + total / B)
    return np.float32(loss)


if __name__ == "__main__":
    # Smoke test with random data (not the reference inputs).
    rng = np.random.default_rng(0)
    ins = {
        "dataX": rng.standard_normal((B, NX), dtype=np.float32),
        "dataY": rng.standard_normal((B, NY), dtype=np.float32),
    }
    for pfx, nin in (("x", NX), ("y", NY)):
        W = pfx.upper() if False else None
        ins[f"W{pfx}0"] = rng.standard_normal((nin, HID), dtype=np.float32) / np.sqrt(nin).astype(np.float32)
        ins[f"b{pfx}0"] = np.zeros(HID, np.float32)
        ins[f"W{pfx}1"] = rng.standard_normal((HID, HID), dtype=np.float32) / np.sqrt(HID).astype(np.float32)
        ins[f"b{pfx}1"] = np.zeros(HID, np.float32)
        ins[f"W{pfx}2"] = rng.standard_normal((HID, EMB), dtype=np.float32) / np.sqrt(HID).astype(np.float32)
        ins[f"b{pfx}2"] = np.zeros(EMB, np.float32)
    print("loss:", kernel(**ins))



# revision 9
# speedup vs baseline: 1.2281x; 1.2281x over previous
"""Distributed flash-style InfoNCE loss kernel for Trainium2 (8 NeuronCores).

Problem: two 3-layer MLP encoders (X and Y) -> [B,B] critic scores ->
InfoNCE MI lower bound:  loss = -(log(B) + mean_i(scores[i,i] - logsumexp_j scores[i,j]))

Sharding: data-parallel over batch rows (1024 rows/core) for both encoders;
zY^T is AllGathered (embed dim is only 128) and each core computes its own
1024 rows of the critic + per-row sum(exp), never materializing the [B,B]
matrix. The exp row-sums come straight out of PSUM via the Activation
engine's fused accumulator over 2048-wide (4-PSUM-bank) tiles; the final
ln() runs on the host, so the device never swaps activation tables.

Overlap structure: the X encoder runs in row groups; each group's critic
chunks (matmul + exp) are emitted interleaved with the next group's encoder
tiles, so the Activation engine's exp pass runs concurrently with encoder
matmuls instead of serially after them. Y-encoder bias+relu runs on the
Activation engine (idle during that phase); X-encoder bias+relu on DVE.
Weights prefetch via the gpsimd SWDGE queue. Matmuls run as float32r.

Per-core output: [128, 16] tile of per-row (pos, sum_exp); the host
computes -(log(B) + mean(pos - log(sum_exp))).
"""

import numpy as np

import concourse.bacc as bacc
import concourse.bass as bass
import concourse.mybir as mybir
import concourse.tile as tile
from concourse.bass_utils import run_bass_kernel_spmd
from concourse.masks import make_identity

# Problem shapes (hardcoded; kernel.py must be self-contained).
B, NX, NY, HID, EMB = 8192, 512, 512, 1024, 128
NCORES = 8
BS = B // NCORES          # 1024 rows per core
P = 128                   # SBUF partitions
MB = BS // P              # 8 row-blocks per core
CW = 2048                 # critic/exp chunk width (4 PSUM banks)
NCC = B // CW             # 4 critic chunks per row-block
G = 2                     # X/Y encoder row groups per core
GR = BS // G              # rows per group
F32 = mybir.dt.float32
F32R = mybir.dt.float32r
BF16 = mybir.dt.bfloat16
AX = mybir.AxisListType
ALU = mybir.AluOpType
ACT = mybir.ActivationFunctionType


def _fr(ap):
    return ap if ap.dtype == F32R else ap.bitcast(F32R)


def _load_bias(nc, pool, name, handle, nblk):
    """[nblk*128] DRAM bias -> [128, nblk] SBUF tile (per-partition layout)."""
    t = pool.tile([P, nblk], F32, name=name, tag=name)
    nc.gpsimd.dma_start(t, handle.ap().rearrange("(m p) -> p m", p=P))
    return t


def _load_weights(nc, wpool, W0, W1, W2, nin_k, prefix):
    """Weight tiles in natural [K, M] layout via the gpsimd SWDGE queue."""
    w0 = []
    for kb in range(nin_k):
        w = wpool.tile([P, HID], F32R, name=f"{prefix}w0_{kb}", tag="w")
        nc.gpsimd.dma_start(w, W0.ap()[kb * P:(kb + 1) * P, :].bitcast(F32R))
        w0.append(w)
    w1 = []
    for kb in range(8):
        w = wpool.tile([P, HID], F32R, name=f"{prefix}w1_{kb}", tag="w")
        nc.gpsimd.dma_start(w, W1.ap()[kb * P:(kb + 1) * P, :].bitcast(F32R))
        w1.append(w)
    w2 = wpool.tile([P, HID], F32R, name=f"{prefix}w2", tag="w")
    for kb in range(8):
        nc.gpsimd.dma_start(w2[:, kb * P:(kb + 1) * P],
                            W2.ap()[kb * P:(kb + 1) * P, :].bitcast(F32R))
    return w0, w1, w2


def _enc_group(nc, pools, weights, biases, data, g, prefix, relu_act, zt, zoff):
    """Emit one encoder row-group (rows g*GR..(g+1)*GR); yields after each
    PSUM-tile unit so the driver can interleave critic emission."""
    const, wpool, xpool, hpool, zpool, inpool, epool, spool, psum = pools
    ident = const["ident"]
    w0, w1, w2 = weights
    b0t, b1t, b2t = biases
    nin_k = len(w0)
    nrb = GR // P

    def _bias_relu(out, in0, b):
        if relu_act:
            nc.scalar.activation(out, in0, ACT.Relu, bias=b)
        else:
            nc.vector.tensor_scalar(out=out, in0=in0, scalar1=b, scalar2=0.0,
                                    op0=ALU.add, op1=ALU.max)

    # ---- transpose input rows -> xt [128, nin_k*GR] (col = kb*GR + r_local)
    xt = xpool.tile([P, nin_k * GR], F32R, name=f"{prefix}xt{g}", tag="xt")
    for rb in range(nrb):
        xin = inpool.tile([P, nin_k * P], F32, name="xin", tag="xin")
        nc.sync.dma_start(xin, data.ap()[(g * nrb + rb) * P:
                                         (g * nrb + rb + 1) * P, :])
        pt = psum.tile([P, CW], F32, name="pt", tag="ps")
        for kb in range(nin_k):
            nc.tensor.transpose(pt[:, kb * 512:kb * 512 + P],
                                xin[:, kb * P:(kb + 1) * P], ident)
        for kb in range(nin_k):
            nc.vector.tensor_copy(
                xt[:, kb * GR + rb * P:kb * GR + (rb + 1) * P],
                pt[:, kb * 512:kb * 512 + P])
        yield

    mpt = CW // GR            # m-blocks per PSUM tile
    # ---- L0: h1T = relu(W0.T @ xT + b0)
    h1 = hpool.tile([P, 8 * GR], F32R, name=f"{prefix}h1_{g}", tag="hx")
    for mq in range(8 // mpt):
        ps = psum.tile([P, CW], F32, name="ps", tag="ps")
        for mi in range(mpt):
            m = mq * mpt + mi
            for kb in range(nin_k):
                nc.tensor.matmul(
                    ps[:, mi * GR:(mi + 1) * GR],
                    _fr(w0[kb][:, m * P:(m + 1) * P]),
                    _fr(xt[:, kb * GR:(kb + 1) * GR]),
                    start=(kb == 0), stop=(kb == nin_k - 1))
        for mi in range(mpt):
            m = mq * mpt + mi
            _bias_relu(h1[:, m * GR:(m + 1) * GR],
                       ps[:, mi * GR:(mi + 1) * GR], b0t[:, m:m + 1])
        yield

    # ---- L1: h2T = relu(W1.T @ h1T + b1)
    h2 = hpool.tile([P, 8 * GR], F32R, name=f"{prefix}h2_{g}", tag="hx")
    for mq in range(8 // mpt):
        ps = psum.tile([P, CW], F32, name="ps", tag="ps")
        for mi in range(mpt):
            m = mq * mpt + mi
            for kb in range(8):
                nc.tensor.matmul(
                    ps[:, mi * GR:(mi + 1) * GR],
                    _fr(w1[kb][:, m * P:(m + 1) * P]),
                    _fr(h1[:, kb * GR:(kb + 1) * GR]),
                    start=(kb == 0), stop=(kb == 7))
        for mi in range(mpt):
            m = mq * mpt + mi
            _bias_relu(h2[:, m * GR:(m + 1) * GR],
                       ps[:, mi * GR:(mi + 1) * GR], b1t[:, m:m + 1])
        yield

    # ---- L2 (linear head): zT slice [128, GR]
    ps = psum.tile([P, CW], F32, name="ps", tag="ps")
    for kb in range(8):
        nc.tensor.matmul(
            ps[:, 0:GR], _fr(w2[:, kb * P:(kb + 1) * P]),
            _fr(h2[:, kb * GR:(kb + 1) * GR]), start=(kb == 0), stop=(kb == 7))
    nc.vector.tensor_scalar(
        out=zt[:, zoff:zoff + GR], in0=ps[:, 0:GR],
        scalar1=b2t[:, 0:1], scalar2=None, op0=ALU.add)
    yield


def _critic_group(nc, pools, g, zxt, zyt, zyall, pos_t, sume):
    """Critic rows for X group g: pos diag + NCC sum(exp) chunks per
    m-block. Yields after each PSUM tile."""
    const, wpool, xpool, hpool, zpool, inpool, epool, spool, psum = pools
    ident = const["ident"]
    nmb = GR // P

    # ---- positive pairs from LOCAL shards: diag(zX_m @ zY_m^T)
    pd = psum.tile([P, CW], F32, name="pd", tag="ps")
    for mi in range(nmb):
        m = g * nmb + mi
        nc.tensor.matmul(pd[:, mi * 512:mi * 512 + P],
                         _fr(zxt[:, mi * P:(mi + 1) * P]),
                         _fr(zyt[:, m * P:(m + 1) * P]),
                         start=True, stop=True)
    for mi in range(nmb):
        m = g * nmb + mi
        dsc = spool.tile([P, P], F32, name="dsc", tag="dsc")
        nc.vector.tensor_mul(dsc, pd[:, mi * 512:mi * 512 + P], ident)
        nc.vector.reduce_sum(pos_t[:, m:m + 1], dsc, axis=AX.X)
    yield

    # ---- critic rows + sum(exp): 2048-wide PSUM tiles, one Exp+accum each.
    for mi in range(nmb):
        m = g * nmb + mi
        for c in range(NCC):
            ps = psum.tile([P, CW], F32, name="ps", tag="ps")
            for q in range(4):
                nc.tensor.matmul(
                    ps[:, q * 512:(q + 1) * 512],
                    _fr(zxt[:, mi * P:(mi + 1) * P]),
                    _fr(zyall[:, c * CW + q * 512:c * CW + (q + 1) * 512]),
                    start=True, stop=True)
            et = epool.tile([P, CW], BF16, name="et", tag="et")
            nc.scalar.activation(
                et, ps, ACT.Exp,
                accum_out=sume[:, m * NCC + c:m * NCC + c + 1])
            yield


def _advance(gen, n):
    """Pull up to n items; True when exhausted."""
    if gen is None:
        return True
    for _ in range(n):
        try:
            next(gen)
        except StopIteration:
            return True
    return False


def build(nrep=1, use_collective=True):
    nc = bacc.Bacc("TRN2", target_bir_lowering=False, debug=False,
                   num_devices=NCORES)

    dX = nc.dram_tensor("dataX", [BS, NX], F32, kind="ExternalInput")
    dY = nc.dram_tensor("dataY", [BS, NY], F32, kind="ExternalInput")
    Wx0 = nc.dram_tensor("Wx0", [NX, HID], F32, kind="ExternalInput")
    bx0 = nc.dram_tensor("bx0", [HID], F32, kind="ExternalInput")
    Wx1 = nc.dram_tensor("Wx1", [HID, HID], F32, kind="ExternalInput")
    bx1 = nc.dram_tensor("bx1", [HID], F32, kind="ExternalInput")
    Wx2 = nc.dram_tensor("Wx2", [HID, EMB], F32, kind="ExternalInput")
    bx2 = nc.dram_tensor("bx2", [EMB], F32, kind="ExternalInput")
    Wy0 = nc.dram_tensor("Wy0", [NY, HID], F32, kind="ExternalInput")
    by0 = nc.dram_tensor("by0", [HID], F32, kind="ExternalInput")
    Wy1 = nc.dram_tensor("Wy1", [HID, HID], F32, kind="ExternalInput")
    by1 = nc.dram_tensor("by1", [HID], F32, kind="ExternalInput")
    Wy2 = nc.dram_tensor("Wy2", [HID, EMB], F32, kind="ExternalInput")
    by2 = nc.dram_tensor("by2", [EMB], F32, kind="ExternalInput")
    # Per-row (pos, sum_exp) pairs: host does the ln().
    out = nc.dram_tensor("out", [P, 2 * MB], F32, kind="ExternalOutput")

    with tile.TileContext(nc) as tc:
        from contextlib import ExitStack
        with ExitStack() as ctx:
            const = ctx.enter_context(tc.tile_pool(name="const", bufs=1))
            wpool = ctx.enter_context(tc.tile_pool(name="wpool", bufs=17))
            xpool = ctx.enter_context(tc.tile_pool(name="xpool", bufs=2))
            hpool = ctx.enter_context(tc.tile_pool(name="hpool", bufs=3))
            zpool = ctx.enter_context(tc.tile_pool(name="zpool", bufs=2))
            inpool = ctx.enter_context(tc.tile_pool(name="inpool", bufs=4))
            epool = ctx.enter_context(tc.tile_pool(name="epool", bufs=2))
            spool = ctx.enter_context(tc.tile_pool(name="spool", bufs=2))
            dram = ctx.enter_context(tc.tile_pool(name="dram", bufs=1, space="DRAM"))
            psum = ctx.enter_context(tc.tile_pool(name="psum", bufs=2, space="PSUM"))

            ident = const.tile([P, P], F32, name="ident", tag="ident")
            make_identity(nc, ident)
            bx0t = _load_bias(nc, const, "bx0t", bx0, 8)
            bx1t = _load_bias(nc, const, "bx1t", bx1, 8)
            bx2t = _load_bias(nc, const, "bx2t", bx2, 1)
            by0t = _load_bias(nc, const, "by0t", by0, 8)
            by1t = _load_bias(nc, const, "by1t", by1, 8)
            by2t = _load_bias(nc, const, "by2t", by2, 1)

            # Preload the exp activation table while the encoders run.
            dummy = const.tile([P, 1], F32, name="dummy", tag="dummy")
            nc.scalar.activation(dummy, ident[:, 0:1], ACT.Exp)

            pools = (
                {"ident": ident}, wpool, xpool, hpool, zpool, inpool, epool,
                spool, psum)

            wy = _load_weights(nc, wpool, Wy0, Wy1, Wy2, NY // P, "y")
            wx = _load_weights(nc, wpool, Wx0, Wx1, Wx2, NX // P, "x")
            by_t = (by0t, by1t, by2t)
            bx_t = (bx0t, bx1t, bx2t)

            for rep in range(nrep):
                # ---- Y encoder (grouped, sequential; relu on Act engine).
                zyt = zpool.tile([P, BS], F32R, name=f"yzt{rep}", tag="yzt")
                for g in range(G):
                    for _ in _enc_group(nc, pools, wy, by_t, dY, g, "y", True,
                                        zyt, g * GR):
                        pass

                zy_bounce = dram.tile([P, BS], F32, name=f"zy_bounce{rep}")
                zy_all = dram.tile([NCORES * P, BS], F32, name=f"zy_all{rep}",
                                   addr_space="Shared" if use_collective else "Local")
                nc.sync.dma_start(zy_bounce.bitcast(F32R), zyt)
                if use_collective:
                    nc.gpsimd.collective_compute(
                        "AllGather", ALU.bypass,
                        replica_groups=[list(range(NCORES))],
                        ins=[zy_bounce.opt()], outs=[zy_all.opt()])
                else:
                    # Sim-only stand-in with the same DMA traffic shape.
                    for r in range(NCORES):
                        nc.sync.dma_start(zy_all[r * P:(r + 1) * P, :],
                                          zy_bounce)

                # All-gathered zY^T: [128, 8192], col = rank*1024 + local row.
                zyall = hpool.tile([P, B], F32R, name="zyall", tag="zyall")
                for r in range(NCORES):
                    nc.sync.dma_start(zyall[:, r * BS:(r + 1) * BS],
                                      zy_all[r * P:(r + 1) * P, :].bitcast(F32R))

                # ---- X encoder groups, critic interleaved one group behind.
                pos_t = spool.tile([P, MB], F32, name="pos_t", tag="pos")
                sume = spool.tile([P, NCC * MB], F32, name="sume", tag="sume")
                zxt = {}

                def _eg(g):
                    zxt[g] = zpool.tile([P, GR], F32R, name=f"xzt{g}",
                                        tag="xzt")
                    yield from _enc_group(nc, pools, wx, bx_t, dX, g, "x",
                                          False, zxt[g], 0)

                egens = [_eg(g) for g in range(G)]
                for _ in egens[0]:
                    pass
                for g in range(G):
                    cgen = _critic_group(nc, pools, g, zxt[g], zyt, zyall,
                                         pos_t, sume)
                    egen = egens[g + 1] if g + 1 < G else None
                    while True:
                        done_c = _advance(cgen, 2)
                        done_e = _advance(egen, 1)
                        if done_c and done_e:
                            break

                tot = spool.tile([P, MB], F32, name="tot", tag="tot")
                for m in range(MB):
                    nc.vector.reduce_sum(tot[:, m:m + 1],
                                         sume[:, m * NCC:(m + 1) * NCC],
                                         axis=AX.X)

                nc.sync.dma_start(out.ap()[:, 0:MB], pos_t)
                nc.sync.dma_start(out.ap()[:, MB:2 * MB], tot)

    nc.compile()
    return nc


_NC_CACHE = None


def _get_nc():
    global _NC_CACHE
    if _NC_CACHE is None:
        _NC_CACHE = build()
    return _NC_CACHE


def kernel(**inputs) -> np.ndarray:
    nc = _get_nc()
    arrs = {k: np.ascontiguousarray(np.asarray(v, dtype=np.float32))
            for k, v in inputs.items()}
    shared = {k: v for k, v in arrs.items() if k not in ("dataX", "dataY")}
    in_maps = []
    for c in range(NCORES):
        m = dict(shared)
        m["dataX"] = np.ascontiguousarray(arrs["dataX"][c * BS:(c + 1) * BS])
        m["dataY"] = np.ascontiguousarray(arrs["dataY"][c * BS:(c + 1) * BS])
        in_maps.append(m)
    res = run_bass_kernel_spmd(nc, in_maps, core_ids=list(range(NCORES)))
    vals = np.stack([res.results[c]["out"] for c in range(NCORES)])  # [8,128,16]
    pos = vals[:, :, :MB].astype(np.float64)
    tot = vals[:, :, MB:].astype(np.float64)
    total = (pos - np.log(tot)).sum()
    loss = -(np.log(np.float64(B)) + total / B)
    return np.float32(loss)


if __name__ == "__main__":
    # Smoke test with random data (not the reference inputs).
    rng = np.random.default_rng(0)
    ins = {
        "dataX": rng.standard_normal((B, NX), dtype=np.float32),
        "dataY": rng.standard_normal((B, NY), dtype=np.float32),
    }
    for pfx, nin in (("x", NX), ("y", NY)):
        ins[f"W{pfx}0"] = rng.standard_normal((nin, HID), dtype=np.float32) / np.sqrt(nin).astype(np.float32)
        ins[f"b{pfx}0"] = np.zeros(HID, np.float32)
        ins[f"W{pfx}1"] = rng.standard_normal((HID, HID), dtype=np.float32) / np.sqrt(HID).astype(np.float32)
        ins[f"b{pfx}1"] = np.zeros(HID, np.float32)
        ins[f"W{pfx}2"] = rng.standard_normal((HID, EMB), dtype=np.float32) / np.sqrt(HID).astype(np.float32)
        ins[f"b{pfx}2"] = np.zeros(EMB, np.float32)
    print("loss:", kernel(**ins))
